# revision 1
# baseline (speedup 1.0000x reference)
"""Cost-volume builder (correlation layer) for Trainium2, 8-core SPMD.

out[b, d, h, w] = (1/sqrt(C)) * sum_c feat1[b,c,h,w] * feat2[b,c,h+dy,w+dx]
for d = (dy+4)*9 + (dx+4), dy,dx in [-4,4]. B,C,H,W = 4,128,192,256.

Sharding: 8 cores = 4 batches x 2 H-halves (96 rows each, feat2 halo +-4).

Per-core algorithm (two 48-row halves):
  Pass 1 (PE): for each r-block (8 feat2 rows) x w-tile (8 outputs wide,
    16-wide feat2 window): matmul lhsT=F2win[C,8x16=128] (stationary,
    FWL-eligible) vs rhs=F1[C,16 h-rows x 8 w =128] -> band tile [128,128]
    in PSUM: band[(j,we), (h,w)] = sum_c F2[c,r0+j,we] * F1[c,h,w].
  Stage (DVE/ACT): PSUM->SBUF fp16 cast copies (4 band tiles per bank).
  Pass 2 (PE): 128 constant one-hot selection matrices Sel[128,81]; per
    (phase t, wl) two PSUM-accumulated matmuls over paired r-blocks pick,
    for every output position, its 81 displacement values:
    out2[(dy,dx), (k,w0)] with h = 8k + t - 8, w = 8*w0 + wl.
  Out-copies (DVE/ACT): [81,192] PSUM->SBUF out tile, then DMA to HBM.
"""

import math

import numpy as np

B, C, H, W = 4, 128, 192, 256
D = 81
NCORES = 8
OH = H // 2            # 96 output rows per core
HQ = 48                # rows per processed half
NRB = 7                # r-blocks per half, 8 rows each, r in [-4, 51]
WT = 32                # w-tiles per row (T=8)
T = 8
WE = 16                # feat2 w-window per tile
F2W = W + 8            # 264, zero-padded W
F2H = OH + 8           # 104 rows incl halo
SCALE = 1.0 / math.sqrt(C)


def _build_sel():
    """[128, 128*81] fp16 one-hot selection matrices, class c=(h_off*8+wl).
    Weight-column order of pass-1 lhsT is (kappa, j): row = kappa*8 + j."""
    sel = np.zeros((128, 128, 81), np.float16)
    for h_off in range(16):
        for wl in range(8):
            cls = h_off * 8 + wl
            for j in range(8):
                dy = j + 4 - h_off
                if -4 <= dy <= 4:
                    for dxh in range(9):  # dxh = dx + 4
                        row = (wl + dxh) * 8 + j
                        col = (dy + 4) * 9 + dxh
                        sel[row, cls, col] = 1.0
    return sel.reshape(128, 128 * 81)


def _emit(tc, f1, f2, selt, out):
    """Emit the Tile program. f1:[C,OH*W] f16, f2:[C,F2H*F2W] f16,
    selt:[C,128*81] f16, out:[D,OH*W] f32 (DRAM APs)."""
    import concourse.bass as bass
    import concourse.mybir as mybir

    dt = mybir.dt
    nc = tc.nc
    MS = bass.MemorySpace

    with (
        tc.tile_pool(name="const", bufs=1) as cpool,
        tc.tile_pool(name="f1p", bufs=1) as f1p,
        tc.tile_pool(name="f2p", bufs=3) as f2p,
        tc.tile_pool(name="stgp", bufs=1) as stgp,
        tc.tile_pool(name="outp", bufs=1) as outp,
        tc.tile_pool(name="ps1", bufs=4, space=MS.PSUM) as ps1,
        tc.tile_pool(name="ps2", bufs=4, space=MS.PSUM) as ps2,
    ):
        selb = cpool.tile([128, 128 * 81], dt.float16)
        nc.sync.dma_start(selb[:, :], selt[:, :])

        for q in range(2):
            # ---- load F1 half: rows h in [-8, 55] (halo zeros baked host-side)
            f1h = f1p.tile([128, 64 * W], dt.float16, tag="f1h")
            nc.sync.dma_start(
                f1h[:, :], f1[:, q * 64 * W : (q + 1) * 64 * W]
            )

            # stage layout: col = cls * 224 + (k*32 + w0), cls = h_off*8 + wl
            stg = stgp.tile([128, 128 * NRB * WT], dt.float16, tag="stg")
            stv2 = stg[:, :].rearrange("p (c t) -> p c t", c=128)
            eng = 0

            # ---- pass 1: band matmuls ----
            for k in range(NRB):
                # slab s = k + NRB*q: [C, 264, 8] = F2 rows [8k-4+48q, 8k+3+48q]
                # transposed host-side so each 16x8 window is contiguous.
                f2s = f2p.tile([128, F2W * 8], dt.float16, tag="f2s")
                s = k + NRB * q
                nc.sync.dma_start(
                    f2s[:, :],
                    f2[:, s * F2W * 8 : (s + 1) * F2W * 8],
                )
                f1v = f1h[:, :].rearrange("p (h x) -> p h x", h=64)
                for g in range(8):  # groups of 4 w-tiles per PSUM bank
                    pt = ps1.tile([128, 512], dt.float32, tag="ps1")
                    for u in range(4):
                        w0 = g * 4 + u
                        lhsT = f2s[:, 64 * w0 : 64 * w0 + 128]     # [128,128]
                        rhs = f1v[:, 8 * k : 8 * k + 16, 8 * w0 : 8 * w0 + T]
                        nc.tensor.matmul(
                            pt[:, u * 128 : (u + 1) * 128],
                            lhsT,
                            rhs,
                            start=True,
                            stop=True,
                        )
                    # psum col = u*128 + cls  ->  stage (cls, t0+u)
                    t0 = k * 32 + g * 4
                    src = pt[:, :].rearrange("p (u c) -> p c u", u=4)
                    dst = stv2[:, :, t0 : t0 + 4]
                    if eng == 0:
                        nc.vector.tensor_copy(dst, src)
                    else:
                        nc.scalar.copy(dst, src)
                    eng ^= 1

            # ---- pass 2: selection matmuls + out copies ----
            outh = outp.tile([81, HQ * W], dt.float32, tag="outh")
            ov = outh[:, :].rearrange(
                "p (a b c d) -> p a b c d", a=6, b=8, c=32, d=8
            )  # h = 8a + b, w = 8c + d
            for t in range(8, 16):
                for wl in range(8):
                    clsA = t * 8 + wl
                    clsB = (t - 8) * 8 + wl
                    p2 = ps2.tile([128, 192], dt.float32, tag="ps2")
                    rhsA = stv2[:, clsA, 0:192]   # [128, 192] tiles k=0..5
                    rhsB = stv2[:, clsB, 32:224]  # [128, 192] tiles k=1..6
                    nc.tensor.matmul(
                        p2[0:81, :],
                        selb[:, clsA * 81 : (clsA + 1) * 81],
                        rhsA,
                        start=True,
                        stop=False,
                    )
                    nc.tensor.matmul(
                        p2[0:81, :],
                        selb[:, clsB * 81 : (clsB + 1) * 81],
                        rhsB,
                        start=False,
                        stop=True,
                    )
                    dst = ov[:, :, t - 8, :, wl]  # [81, 6, 32]
                    src = p2[0:81, :].rearrange("p (a b) -> p a b", a=6)
                    if eng == 0:
                        nc.vector.tensor_copy(dst, src)
                    else:
                        nc.scalar.copy(dst, src)
                    eng ^= 1

            nc.sync.dma_start(
                out[:, q * HQ * W : (q + 1) * HQ * W], outh[:, :]
            )


def _build_nc():
    import concourse.mybir as mybir
    import concourse.tile as tile
    from concourse import bacc

    dt = mybir.dt
    nc = bacc.Bacc("TRN2", target_bir_lowering=False, debug=False)
    f1 = nc.dram_tensor("f1", [C, 2 * 64 * W], dt.float16, kind="ExternalInput")
    f2 = nc.dram_tensor(
        "f2", [C, 2 * NRB * F2W * 8], dt.float16, kind="ExternalInput"
    )
    selt = nc.dram_tensor("sel", [C, 128 * 81], dt.float16, kind="ExternalInput")
    out = nc.dram_tensor("out", [D, OH * W], dt.float32, kind="ExternalOutput")
    with tile.TileContext(nc) as tc:
        _emit(tc, f1[:, :], f2[:, :], selt[:, :], out[:, :])
    nc.finalize()
    return nc


def _shard_inputs(feat1, feat2):
    sel = _build_sel()
    in_maps = []
    for core in range(NCORES):
        b, half = core // 2, core % 2
        h0 = half * OH
        f1s = np.zeros((C, 2, 64, W), np.float16)
        for q in range(2):
            glo, ghi = q * HQ - 8, q * HQ + 56   # rows -8..55 of this half
            slo, shi = max(glo, 0), min(ghi, OH)
            f1s[:, q, slo - glo : shi - glo, :] = (
                feat1[b, :, h0 + slo : h0 + shi, :] * SCALE
            ).astype(np.float16)
        f2pad = np.zeros((C, F2H, F2W), np.float16)
        lo, hi = h0 - 4, h0 + OH + 4
        slo, shi = max(lo, 0), min(hi, H)
        f2pad[:, slo - lo : shi - lo, 4 : 4 + W] = feat2[b, :, slo:shi, :].astype(
            np.float16
        )
        # slabs: s = k + NRB*q -> F2 rows [8k+48q, 8k+48q+8), transposed to
        # [C, w, r] so each (16 w x 8 r) matmul weight window is contiguous.
        slabs = np.zeros((C, 2 * NRB, F2W, 8), np.float16)
        for s in range(2 * NRB):
            base = 8 * (s % NRB) + HQ * (s // NRB)
            slabs[:, s] = f2pad[:, base : base + 8, :].transpose(0, 2, 1)
        in_maps.append(
            {
                "f1": np.ascontiguousarray(f1s.reshape(C, 2 * 64 * W)),
                "f2": np.ascontiguousarray(slabs.reshape(C, -1)),
                "sel": sel,
            }
        )
    return in_maps


def kernel(feat1, feat2):
    feat1 = np.asarray(feat1, dtype=np.float32)
    feat2 = np.asarray(feat2, dtype=np.float32)
    from concourse.bass_utils import run_bass_kernel_spmd

    nc = _build_nc()
    in_maps = _shard_inputs(feat1, feat2)
    res = run_bass_kernel_spmd(nc, in_maps, list(range(NCORES)))
    full = np.zeros((B, D, H, W), np.float32)
    for core in range(NCORES):
        b, half = core // 2, core % 2
        full[b, :, half * OH : (half + 1) * OH, :] = res.results[core][
            "out"
        ].reshape(D, OH, W)
    return full



# revision 5
# speedup vs baseline: 1.1324x; 1.1324x over previous
"""Cost-volume builder (correlation layer) for Trainium2, 8-core SPMD.

out[b, d, h, w] = (1/sqrt(C)) * sum_c feat1[b,c,h,w] * feat2[b,c,h+dy,w+dx]
for d = (dy+4)*9 + (dx+4), dy,dx in [-4,4]. B,C,H,W = 4,128,192,256.

Sharding: 8 cores = 4 batches x 2 H-halves (96 rows each, feat2 halo +-4).

Per-core algorithm (two 48-row halves over a single resident f1):
  f1 resident: [C, 112 rows x W] fp16 (global h -8..104, halo rows
    memset to zero on device; valid 96 rows DMAed in 6 chunks).
  Pass 1 (PE): for each r-block k (8 feat2 rows, slab g = 6q+k of 13)
    x w-tile (8 outputs wide, 16-wide feat2 window): matmul
    lhsT=F2win[C,8x16=128] vs rhs=F1[C,16 h-rows x 8 w] -> band tile
    in PSUM: band[(we,j), (h,w)] = sum_c F2[c,r0+j,we] * F1[c,h,w].
    Boundary blocks (k=0 local h_off 8..15 only, k=6 local h_off 0..7
    only) run half-width rhs: the other rows belong to the adjacent
    half (or are zero padding) and are never read by pass 2.
  Stage (DVE/ACT): PSUM->SBUF fp16 cast copies.
  Pass 2 (PE): 128 constant one-hot selection matrices Sel[128,81]; per
    (phase t, wl) two PSUM-accumulated matmuls over paired r-blocks pick
    each output position's 81 displacement values.
  Out (DVE/ACT + DMA): per phase t, 8 wl-copies cast PSUM fp32 ->
    [81, 6x256] fp16 tile, then one strided store (gpsimd DMA queue so
    stores never head-of-line block loads on the sync queue).
"""

import math

import numpy as np

B, C, H, W = 4, 128, 192, 256
D = 81
NCORES = 8
OH = H // 2            # 96 output rows per core
HQ = 48                # rows per processed half
NRB = 7                # r-blocks per half, 8 rows each
NSLAB = 13             # unique f2 slabs per core (g = 6q + k)
WT = 32                # w-tiles per row (T=8)
T = 8
WE = 16                # feat2 w-window per tile
F2W = W + 8            # 264, zero-padded W
F1R = 112              # resident f1 rows: global h in [-8, 104)
SCALE = 1.0 / math.sqrt(C)


def _build_sel():
    """[128, 128*81] fp16 one-hot selection matrices, class c=(h_off*8+wl).
    Weight-column order of pass-1 lhsT is (kappa, j): row = kappa*8 + j."""
    sel = np.zeros((128, 128, 81), np.float16)
    for h_off in range(16):
        for wl in range(8):
            cls = h_off * 8 + wl
            for j in range(8):
                dy = j + 4 - h_off
                if -4 <= dy <= 4:
                    for dxh in range(9):  # dxh = dx + 4
                        row = (wl + dxh) * 8 + j
                        col = (dy + 4) * 9 + dxh
                        sel[row, cls, col] = 1.0
    return sel.reshape(128, 128 * 81)


def _emit(tc, f1, f2, selt, out):
    """Emit the Tile program. f1:[C,96*W] f16, f2:[C,13*F2W*8] f16,
    selt:[C,128*81] f16, out:[D,OH*W] f16 (DRAM APs)."""
    import concourse.bass as bass
    import concourse.mybir as mybir

    dt = mybir.dt
    nc = tc.nc
    MS = bass.MemorySpace

    with (
        tc.tile_pool(name="const", bufs=1) as cpool,
        tc.tile_pool(name="f1p", bufs=1) as f1p,
        tc.tile_pool(name="f2p", bufs=6) as f2p,
        tc.tile_pool(name="stgp", bufs=1) as stgp,
        tc.tile_pool(name="outp", bufs=3) as outp,
        tc.tile_pool(name="ps1", bufs=4, space=MS.PSUM) as ps1,
        tc.tile_pool(name="ps2", bufs=4, space=MS.PSUM) as ps2,
    ):
        # ---- resident f1: rows 0..111 = global h -8..104 ----
        f1h = f1p.tile([128, F1R * W], dt.float16, tag="f1h")
        nc.gpsimd.memset(f1h[:, 0 : 8 * W], 0.0)
        nc.gpsimd.memset(f1h[:, 104 * W : F1R * W], 0.0)
        f1v = f1h[:, :].rearrange("p (h x) -> p h x", h=F1R)

        selb = cpool.tile([128, 128 * 81], dt.float16)

        # f2 slab tiles, allocated on first use, kept by slab index g
        slabs = {}

        def load_slab(g):
            t = f2p.tile([128, F2W * 8], dt.float16, tag="f2s")
            nc.sync.dma_start(t[:, :], f2[:, g * F2W * 8 : (g + 1) * F2W * 8])
            slabs[g] = t

        def load_f1(c):  # chunk c: f1 rows 16c..16c+16 -> tile rows +8
            nc.sync.dma_start(
                f1h[:, (8 + 16 * c) * W : (24 + 16 * c) * W],
                f1[:, 16 * c * W : (16 * c + 16) * W],
            )

        # Load order tuned so pass-1 q=0 r-blocks arrive just in time,
        # sel lands before pass-2 q=0, and the rest streams behind.
        for g in range(4):
            load_slab(g)
            load_f1(g)
        for g in range(4, 7):
            load_slab(g)
        nc.sync.dma_start(selb[:, :], selt[:, :])
        load_f1(4)
        load_f1(5)
        for g in range(7, NSLAB):
            load_slab(g)

        ov = out[:, :].rearrange(
            "d (q a tt b c) -> d q a tt b c", q=2, a=6, tt=8, b=32
        )  # h = 48q + 8a + tt, w = 8b + c
        eng = 0

        for q in range(2):
            # stage layout: col = cls * 224 + (k*32 + w0), cls = h_off*8 + wl
            stg = stgp.tile([128, 128 * NRB * WT], dt.float16, tag="stg")
            stv2 = stg[:, :].rearrange("p (c t) -> p c t", c=128)

            # ---- pass 1: band matmuls ----
            for k in range(NRB):
                f2s = slabs[6 * q + k]
                r0 = 48 * q + 8 * k
                # boundary trim: k=0 keeps h_off 8..15, k=6 keeps 0..7
                if k == 0:
                    rlo, rhi, cls0 = r0 + 8, r0 + 16, 64
                elif k == NRB - 1:
                    rlo, rhi, cls0 = r0, r0 + 8, 0
                else:
                    rlo, rhi, cls0 = r0, r0 + 16, None
                mw = rhi - rlo  # moving rows (8 or 16)
                for g in range(8):  # groups of 4 w-tiles per PSUM bank
                    pt = ps1.tile([128, mw * 8 * 4], dt.float32, tag="ps1")
                    for u in range(4):
                        w0 = g * 4 + u
                        lhsT = f2s[:, 64 * w0 : 64 * w0 + 128]     # [128,128]
                        rhs = f1v[:, rlo:rhi, 8 * w0 : 8 * w0 + T]
                        nmv = mw * 8
                        nc.tensor.matmul(
                            pt[:, u * nmv : (u + 1) * nmv],
                            lhsT,
                            rhs,
                            start=True,
                            stop=True,
                        )
                    t0 = k * 32 + g * 4
                    src = pt[:, :].rearrange("p (u c) -> p c u", u=4)
                    if cls0 is None:
                        dst = stv2[:, :, t0 : t0 + 4]
                    else:
                        dst = stv2[:, cls0 : cls0 + 64, t0 : t0 + 4]
                    if eng == 0:
                        nc.vector.tensor_copy(dst, src)
                    else:
                        nc.scalar.copy(dst, src)
                    eng ^= 1

            # ---- pass 2: selection matmuls + per-phase out stores ----
            for t in range(8, 16):
                ot = outp.tile([81, 6 * W], dt.float16, tag="outt")
                otv = ot[:, :].rearrange("p (a b c) -> p a b c", a=6, b=32)
                for wl in range(8):
                    clsA = t * 8 + wl
                    clsB = (t - 8) * 8 + wl
                    p2 = ps2.tile([128, 192], dt.float32, tag="ps2")
                    rhsA = stv2[:, clsA, 0:192]   # [128, 192] tiles k=0..5
                    rhsB = stv2[:, clsB, 32:224]  # [128, 192] tiles k=1..6
                    nc.tensor.matmul(
                        p2[0:81, :],
                        selb[:, clsA * 81 : (clsA + 1) * 81],
                        rhsA,
                        start=True,
                        stop=False,
                    )
                    nc.tensor.matmul(
                        p2[0:81, :],
                        selb[:, clsB * 81 : (clsB + 1) * 81],
                        rhsB,
                        start=False,
                        stop=True,
                    )
                    dst = otv[:, :, :, wl]  # [81, 6, 32]
                    src = p2[0:81, :].rearrange("p (a b) -> p a b", a=6)
                    if eng == 0:
                        nc.vector.tensor_copy(dst, src)
                    else:
                        nc.scalar.copy(dst, src)
                    eng ^= 1
                nc.gpsimd.dma_start(ov[:, q, :, t - 8, :, :], otv[:, :, :, :])


def _build_nc():
    import concourse.mybir as mybir
    import concourse.tile as tile
    from concourse import bacc

    dt = mybir.dt
    nc = bacc.Bacc("TRN2", target_bir_lowering=False, debug=False)
    f1 = nc.dram_tensor("f1", [C, OH * W], dt.float16, kind="ExternalInput")
    f2 = nc.dram_tensor(
        "f2", [C, NSLAB * F2W * 8], dt.float16, kind="ExternalInput"
    )
    selt = nc.dram_tensor("sel", [C, 128 * 81], dt.float16, kind="ExternalInput")
    out = nc.dram_tensor("out", [D, OH * W], dt.float16, kind="ExternalOutput")
    with tile.TileContext(nc) as tc:
        _emit(tc, f1[:, :], f2[:, :], selt[:, :], out[:, :])
    nc.finalize()
    return nc


def _shard_inputs(feat1, feat2):
    sel = _build_sel()
    in_maps = []
    for core in range(NCORES):
        b, half = core // 2, core % 2
        h0 = half * OH
        f1s = (feat1[b, :, h0 : h0 + OH, :] * SCALE).astype(np.float16)
        f2pad = np.zeros((C, OH + 8, F2W), np.float16)
        lo, hi = h0 - 4, h0 + OH + 4
        slo, shi = max(lo, 0), min(hi, H)
        f2pad[:, slo - lo : shi - lo, 4 : 4 + W] = feat2[b, :, slo:shi, :].astype(
            np.float16
        )
        # slab g -> f2pad rows [8g, 8g+8) (= global rows 8g-4..8g+4),
        # transposed to [C, w, r] so each (16 w x 8 r) weight window is
        # contiguous.
        slabs = np.zeros((C, NSLAB, F2W, 8), np.float16)
        for g in range(NSLAB):
            slabs[:, g] = f2pad[:, 8 * g : 8 * g + 8, :].transpose(0, 2, 1)
        in_maps.append(
            {
                "f1": np.ascontiguousarray(f1s.reshape(C, OH * W)),
                "f2": np.ascontiguousarray(slabs.reshape(C, -1)),
                "sel": sel,
            }
        )
    return in_maps


def kernel(feat1, feat2):
    feat1 = np.asarray(feat1, dtype=np.float32)
    feat2 = np.asarray(feat2, dtype=np.float32)
    from concourse.bass_utils import run_bass_kernel_spmd

    nc = _build_nc()
    in_maps = _shard_inputs(feat1, feat2)
    res = run_bass_kernel_spmd(nc, in_maps, list(range(NCORES)))
    full = np.zeros((B, D, H, W), np.float32)
    for core in range(NCORES):
        b, half = core // 2, core % 2
        full[b, :, half * OH : (half + 1) * OH, :] = (
            res.results[core]["out"].astype(np.float32).reshape(D, OH, W)
        )
    return full


# revision 11
# speedup vs baseline: 1.2510x; 1.1047x over previous
"""Cost-volume builder (correlation layer) for Trainium2, 8-core SPMD.

out[b, d, h, w] = (1/sqrt(C)) * sum_c feat1[b,c,h,w] * feat2[b,c,h+dy,w+dx]
for d = (dy+4)*9 + (dx+4), dy,dx in [-4,4]. B,C,H,W = 4,128,192,256.

Sharding: 8 cores = 4 batches x 2 H-halves (96 rows each, feat2 halo +-4).

Per-core algorithm (two 48-row halves over a single resident f1):
  f1 resident: [C, 112 rows x W] fp16 (global h -8..104, halo rows
    memset to zero on device; valid 96 rows DMAed in 6 chunks).
  Pass 1 (PE): per r-block k (8 feat2 rows, slab g = 6q+k of 13) x
    w-tile (8 outputs wide, 16-wide feat2 window): matmul
    lhsT=F2win[C,8x16=128] vs rhs=F1[C,16 h-rows x 8 w] -> band tile
    in PSUM: band[(we,j), (h,w)] = sum_c F2[c,r0+j,we] * F1[c,h,w].
    Boundary blocks (k=0 keeps local h_off 8..15, k=6 keeps 0..7) run
    half-width rhs; the dropped rows belong to the adjacent half (or
    are zero padding) and are never read by pass 2.
    8 matmuls share one 2-bank PSUM tile so the stage copy is 1 op per
    1024 cols (per-op PSUM access latency dominates copy cost).
  Stage (DVE/ACT/POOL rotate): PSUM->SBUF fp16 cast copies.
  Pass 2 (PE): 128 constant one-hot selection matrices Sel[128,81]; per
    (phase t, wl) two PSUM-accumulated matmuls over paired r-blocks pick
    each output position's 81 displacement values.
  Out (3-engine rotate + DMA): per (t, wl) copy PSUM fp32 -> phase tile
    [81, 6x256] fp32, then one strided store per phase on the sync
    hardware DMA queue (enqueued after all loads, so no blocking).
"""

import math

import numpy as np

B, C, H, W = 4, 128, 192, 256
D = 81
NCORES = 8
OH = H // 2            # 96 output rows per core
HQ = 48                # rows per processed half
NRB = 7                # r-blocks per half, 8 rows each
NSLAB = 13             # unique f2 slabs per core (g = 6q + k)
WT = 32                # w-tiles per row (T=8)
T = 8
WE = 16                # feat2 w-window per tile
F2W = W + 8            # 264, zero-padded W
F1R = 112              # resident f1 rows: global h in [-8, 104)
SCALE = 1.0 / math.sqrt(C)


def _build_sel():
    """[128, 128*81] fp16 one-hot selection matrices, class c=(h_off*8+wl).
    Weight-column order of pass-1 lhsT is (kappa, j): row = kappa*8 + j."""
    sel = np.zeros((128, 128, 81), np.float16)
    for h_off in range(16):
        for wl in range(8):
            cls = h_off * 8 + wl
            for j in range(8):
                dy = j + 4 - h_off
                if -4 <= dy <= 4:
                    for dxh in range(9):  # dxh = dx + 4
                        row = (wl + dxh) * 8 + j
                        col = (dy + 4) * 9 + dxh
                        sel[row, cls, col] = 1.0
    return sel.reshape(128, 128 * 81)


def _emit(tc, f1, f2, selt, out):
    """Emit the Tile program. f1:[C,96*W] f16, f2:[C,13*F2W*8] f16,
    selt:[C,128*81] f16, out:[D,OH*W] f32 (DRAM APs)."""
    import concourse.bass as bass
    import concourse.mybir as mybir

    dt = mybir.dt
    nc = tc.nc
    MS = bass.MemorySpace

    # GPSIMD cannot access PSUM, so PSUM->SBUF copies rotate DVE/ACT only.
    copy_engines = [nc.vector.tensor_copy, nc.scalar.copy]
    eng = [0]

    def copy(dst, src):
        copy_engines[eng[0]](dst, src)
        eng[0] = (eng[0] + 1) % 2

    with (
        tc.tile_pool(name="const", bufs=1) as cpool,
        tc.tile_pool(name="f1p", bufs=1) as f1p,
        tc.tile_pool(name="f2p", bufs=4) as f2p,
        tc.tile_pool(name="stgp", bufs=1) as stgp,
        tc.tile_pool(name="outp", bufs=3) as outp,
        tc.tile_pool(name="ps", bufs=4, space=MS.PSUM) as psp,
    ):
        # ---- resident f1: rows 0..111 = global h -8..104 ----
        f1h = f1p.tile([128, F1R * W], dt.float16, tag="f1h")
        nc.gpsimd.memset(f1h[:, 0 : 8 * W], 0.0)
        nc.gpsimd.memset(f1h[:, 104 * W : F1R * W], 0.0)
        f1v = f1h[:, :].rearrange("p (h x) -> p h x", h=F1R)

        selb = cpool.tile([128, 128 * 81], dt.float16)

        # f2 slab pairs: tile i holds slabs 2i, 2i+1 (last holds g=12)
        pair_tiles = {}

        def load_pair(i):
            n = 1 if i == 6 else 2
            t = f2p.tile([128, n * F2W * 8], dt.float16, tag="f2s")
            nc.sync.dma_start(
                t[:, :], f2[:, 2 * i * F2W * 8 : (2 * i + n) * F2W * 8]
            )
            pair_tiles[i] = t

        def slab(g):
            return pair_tiles[g // 2][:, (g % 2) * F2W * 8 : (g % 2 + 1) * F2W * 8]

        def load_f1(c):  # chunk c: f1 rows 16c..16c+16 -> tile rows +8
            nc.sync.dma_start(
                f1h[:, (8 + 16 * c) * W : (24 + 16 * c) * W],
                f1[:, 16 * c * W : (16 * c + 16) * W],
            )

        # Load order tuned so pass-1 q=0 r-blocks arrive just in time,
        # sel lands before pass-2 q=0, and the rest streams behind.
        load_pair(0)
        load_f1(0)
        load_pair(1)
        load_f1(1)
        load_pair(2)
        load_f1(2)
        load_f1(3)
        load_pair(3)
        nc.sync.dma_start(selb[:, :], selt[:, :])
        load_f1(4)
        load_f1(5)
        load_pair(4)
        load_pair(5)
        load_pair(6)

        ov = out[:, :].rearrange(
            "d (q a tt b c) -> d q a tt b c", q=2, a=6, tt=8, b=32
        )  # h = 48q + 8a + tt, w = 8b + c

        for q in range(2):
            # stage layout: col = cls * 224 + (k*32 + w0), cls = h_off*8 + wl
            stg = stgp.tile([128, 128 * NRB * WT], dt.float16, tag="stg")
            stv2 = stg[:, :].rearrange("p (c t) -> p c t", c=128)

            # ---- pass 1: band matmuls ----
            for k in range(NRB):
                f2s = slab(6 * q + k)
                r0 = 48 * q + 8 * k
                # boundary trim: k=0 keeps h_off 8..15, k=6 keeps 0..7
                if k == 0:
                    rlo, rhi, cls0 = r0 + 8, r0 + 16, 64
                elif k == NRB - 1:
                    rlo, rhi, cls0 = r0, r0 + 8, 0
                else:
                    rlo, rhi, cls0 = r0, r0 + 16, None
                mw = rhi - rlo  # moving rows (8 or 16)
                nmv = mw * 8
                for g in range(4):  # groups of 8 w-tiles per 2-bank PSUM tile
                    pt = psp.tile([128, nmv * 8], dt.float32, tag="ps1")
                    for u in range(8):
                        w0 = g * 8 + u
                        lhsT = f2s[:, 64 * w0 : 64 * w0 + 128]     # [128,128]
                        rhs = f1v[:, rlo:rhi, 8 * w0 : 8 * w0 + T]
                        nc.tensor.matmul(
                            pt[:, u * nmv : (u + 1) * nmv],
                            lhsT,
                            rhs,
                            start=True,
                            stop=True,
                        )
                    t0 = k * 32 + g * 8
                    src = pt[:, :].rearrange("p (u c) -> p c u", u=8)
                    if cls0 is None:
                        dst = stv2[:, :, t0 : t0 + 8]
                    else:
                        dst = stv2[:, cls0 : cls0 + 64, t0 : t0 + 8]
                    copy(dst, src)

            # ---- pass 2: selection matmuls + per-phase out stores ----
            for t in range(8, 16):
                ot = outp.tile([81, 6 * W], dt.float32, tag="outt")
                otv = ot[:, :].rearrange("p (a b c) -> p a b c", a=6, b=32)
                for gr in range(2):  # 4 wl per PSUM tile -> 1 grouped copy
                    p2 = psp.tile([128, 1024], dt.float32, tag="ps1")
                    for s in range(4):
                        wl = 4 * gr + s
                        clsA = t * 8 + wl
                        clsB = (t - 8) * 8 + wl
                        dst2 = p2[0:81, 256 * s : 256 * s + 192]
                        nc.tensor.matmul(
                            dst2,
                            selb[:, clsA * 81 : (clsA + 1) * 81],
                            stv2[:, clsA, 0:192],   # tiles k=0..5
                            start=True,
                            stop=False,
                        )
                        nc.tensor.matmul(
                            dst2,
                            selb[:, clsB * 81 : (clsB + 1) * 81],
                            stv2[:, clsB, 32:224],  # tiles k=1..6
                            start=False,
                            stop=True,
                        )
                    dst = otv[:, :, :, 4 * gr : 4 * gr + 4]  # [81, 6, 32, 4]
                    src = (
                        p2[0:81, :]
                        .rearrange("p (s x) -> p s x", s=4)[:, :, 0:192]
                        .rearrange("p s (a b) -> p a b s", a=6)
                    )
                    copy(dst, src)
                nc.sync.dma_start(ov[:, q, :, t - 8, :, :], otv[:, :, :, :])


def _build_nc():
    import concourse.mybir as mybir
    import concourse.tile as tile
    from concourse import bacc

    dt = mybir.dt
    nc = bacc.Bacc("TRN2", target_bir_lowering=False, debug=False)
    f1 = nc.dram_tensor("f1", [C, OH * W], dt.float16, kind="ExternalInput")
    f2 = nc.dram_tensor(
        "f2", [C, NSLAB * F2W * 8], dt.float16, kind="ExternalInput"
    )
    selt = nc.dram_tensor("sel", [C, 128 * 81], dt.float16, kind="ExternalInput")
    out = nc.dram_tensor("out", [D, OH * W], dt.float32, kind="ExternalOutput")
    with tile.TileContext(nc) as tc:
        _emit(tc, f1[:, :], f2[:, :], selt[:, :], out[:, :])
    nc.finalize()
    return nc


def _shard_inputs(feat1, feat2):
    sel = _build_sel()
    in_maps = []
    for core in range(NCORES):
        b, half = core // 2, core % 2
        h0 = half * OH
        f1s = (feat1[b, :, h0 : h0 + OH, :] * SCALE).astype(np.float16)
        f2pad = np.zeros((C, OH + 8, F2W), np.float16)
        lo, hi = h0 - 4, h0 + OH + 4
        slo, shi = max(lo, 0), min(hi, H)
        f2pad[:, slo - lo : shi - lo, 4 : 4 + W] = feat2[b, :, slo:shi, :].astype(
            np.float16
        )
        # slab g -> f2pad rows [8g, 8g+8) (= global rows 8g-4..8g+4),
        # transposed to [C, w, r] so each (16 w x 8 r) weight window is
        # contiguous.
        slabs = np.zeros((C, NSLAB, F2W, 8), np.float16)
        for g in range(NSLAB):
            slabs[:, g] = f2pad[:, 8 * g : 8 * g + 8, :].transpose(0, 2, 1)
        in_maps.append(
            {
                "f1": np.ascontiguousarray(f1s.reshape(C, OH * W)),
                "f2": np.ascontiguousarray(slabs.reshape(C, -1)),
                "sel": sel,
            }
        )
    return in_maps


def kernel(feat1, feat2):
    feat1 = np.asarray(feat1, dtype=np.float32)
    feat2 = np.asarray(feat2, dtype=np.float32)
    from concourse.bass_utils import run_bass_kernel_spmd

    nc = _build_nc()
    in_maps = _shard_inputs(feat1, feat2)
    res = run_bass_kernel_spmd(nc, in_maps, list(range(NCORES)))
    full = np.zeros((B, D, H, W), np.float32)
    for core in range(NCORES):
        b, half = core // 2, core % 2
        full[b, :, half * OH : (half + 1) * OH, :] = (
            res.results[core]["out"].astype(np.float32).reshape(D, OH, W)
        )
    return full


# revision 22
# speedup vs baseline: 1.3393x; 1.0705x over previous
"""Cost-volume builder (correlation layer) for Trainium2, 8-core SPMD.

out[b, d, h, w] = (1/sqrt(C)) * sum_c feat1[b,c,h,w] * feat2[b,c,h+dy,w+dx]
for d = (dy+4)*9 + (dx+4), dy,dx in [-4,4]. B,C,H,W = 4,128,192,256.

Sharding: 8 cores = 4 batches x 2 H-halves (96 rows each, feat2 halo +-4).

Per-core algorithm (two 48-row halves over a single resident f1):
  f1 resident: [C, 112 rows x W] fp16 (global h -8..104, halo rows
    memset to zero on device; valid 96 rows DMAed in 6 chunks).
  Pass 1 (PE): per r-block k (8 feat2 rows, slab g = 6q+k of 13) x
    w-tile (8 outputs wide, 16-wide feat2 window): matmul
    lhsT=F2win[C,8x16=128] vs rhs=F1[C,16 h-rows x 8 w] -> band tile
    in PSUM: band[(we,j), (h,w)] = sum_c F2[c,r0+j,we] * F1[c,h,w].
    Boundary blocks (k=0 keeps local h_off 8..15, k=6 keeps 0..7) run
    half-width rhs; the dropped rows belong to the adjacent half (or
    are zero padding) and are never read by pass 2.
    8 matmuls share one 2-bank PSUM tile so the stage copy is 1 op per
    1024 cols (per-op PSUM access latency dominates copy cost).
  Stage (DVE/ACT/POOL rotate): PSUM->SBUF fp16 cast copies.
  Pass 2 (PE): 128 constant one-hot selection matrices Sel[128,81]; per
    (phase t, wl) two PSUM-accumulated matmuls over paired r-blocks pick
    each output position's 81 displacement values.
  Out (3-engine rotate + DMA): per (t, wl) copy PSUM fp32 -> phase tile
    [81, 6x256] fp32, then one strided store per phase on the sync
    hardware DMA queue (enqueued after all loads, so no blocking).
"""

import math

import numpy as np

B, C, H, W = 4, 128, 192, 256
D = 81
NCORES = 8
OH = H // 2            # 96 output rows per core
HQ = 48                # rows per processed half
NRB = 7                # r-blocks per half, 8 rows each
NSLAB = 13             # unique f2 slabs per core (g = 6q + k)
WT = 32                # w-tiles per row (T=8)
T = 8
WE = 16                # feat2 w-window per tile
F2W = W + 8            # 264, zero-padded W
F1R = 112              # resident f1 rows: global h in [-8, 104)
SCALE = 1.0 / math.sqrt(C)


def _sel_col(h_off, wl):
    """Column block of class (h_off, wl) in the grouped sel layout:
    group 0 = h_off % 8 in 0..3 (pass-2 phases t=8..11), group 1 = rest,
    so each phase-half's matrices are one contiguous 1.33MB DMA."""
    grp = (h_off % 8) // 4
    hh = (h_off // 8) * 4 + (h_off % 4)
    return (grp * 64 + hh * 8 + wl) * 81


def _build_sel():
    """[128, 128*81] fp16 one-hot selection matrices, grouped layout.
    Weight-column order of pass-1 lhsT is (kappa, j): row = kappa*8 + j."""
    sel = np.zeros((128, 128 * 81), np.float16)
    for h_off in range(16):
        for wl in range(8):
            c0 = _sel_col(h_off, wl)
            for j in range(8):
                dy = j + 4 - h_off
                if -4 <= dy <= 4:
                    for dxh in range(9):  # dxh = dx + 4
                        row = (wl + dxh) * 8 + j
                        sel[row, c0 + (dy + 4) * 9 + dxh] = 1.0
    return sel


def _emit(tc, f1, f2, selt, out):
    """Emit the Tile program. f1:[C,96*W] f16, f2:[C,13*F2W*8] f16,
    selt:[C,128*81] f16, out:[D,OH*W] f32 (DRAM APs)."""
    import concourse.bass as bass
    import concourse.mybir as mybir

    dt = mybir.dt
    nc = tc.nc
    MS = bass.MemorySpace

    # GPSIMD cannot access PSUM, so PSUM->SBUF copies rotate DVE/ACT only.
    copy_engines = [nc.vector.tensor_copy, nc.scalar.copy]
    eng = [0]

    def copy(dst, src):
        copy_engines[eng[0]](dst, src)
        eng[0] = (eng[0] + 1) % 2

    with (
        tc.tile_pool(name="const", bufs=1) as cpool,
        tc.tile_pool(name="f1p", bufs=1) as f1p,
        tc.tile_pool(name="f2p", bufs=4) as f2p,
        tc.tile_pool(name="stgp", bufs=1) as stgp,
        tc.tile_pool(name="outp", bufs=3) as outp,
        tc.tile_pool(name="ps", bufs=4, space=MS.PSUM) as psp,
    ):
        # ---- resident f1: rows 0..111 = global h -8..104 ----
        f1h = f1p.tile([128, F1R * W], dt.float16, tag="f1h")
        nc.gpsimd.memset(f1h[:, 0 : 8 * W], 0.0)
        nc.gpsimd.memset(f1h[:, 104 * W : F1R * W], 0.0)
        f1v = f1h[:, :].rearrange("p (h x) -> p h x", h=F1R)

        selb = cpool.tile([128, 128 * 81], dt.float16)

        # f2 slab pairs: tile i holds slabs 2i, 2i+1 (last holds g=12)
        pair_tiles = {}

        def load_pair(i):
            n = 1 if i == 6 else 2
            t = f2p.tile([128, n * F2W * 8], dt.float16, tag="f2s")
            nc.sync.dma_start(
                t[:, :], f2[:, 2 * i * F2W * 8 : (2 * i + n) * F2W * 8]
            )
            pair_tiles[i] = t

        def slab(g):
            return pair_tiles[g // 2][:, (g % 2) * F2W * 8 : (g % 2 + 1) * F2W * 8]

        def load_f1(c):  # chunk c: f1 rows 16c..16c+16 -> tile rows +8
            nc.sync.dma_start(
                f1h[:, (8 + 16 * c) * W : (24 + 16 * c) * W],
                f1[:, 16 * c * W : (16 * c + 16) * W],
            )

        # Load order tuned so pass-1 q=0 r-blocks arrive just in time and
        # the two sel halves land before pass-2 q=0 phases t=8 / t=12.
        HS = 64 * 81
        load_pair(0)
        load_f1(0)
        load_pair(1)
        load_f1(1)
        nc.sync.dma_start(selb[:, 0:HS], selt[:, 0:HS])
        load_pair(2)
        load_f1(2)
        load_f1(3)
        load_pair(3)
        nc.sync.dma_start(selb[:, HS : 2 * HS], selt[:, HS : 2 * HS])
        load_f1(4)
        load_pair(4)
        load_f1(5)
        load_pair(5)
        load_pair(6)

        for q in range(2):
            # stage layout: col = cls * 224 + (k*32 + w0), cls = h_off*8 + wl
            stg = stgp.tile([128, 128 * NRB * WT], dt.float16, tag="stg")
            stv2 = stg[:, :].rearrange("p (c t) -> p c t", c=128)

            # ---- pass 1: band matmuls ----
            for k in range(NRB):
                f2s = slab(6 * q + k)
                r0 = 48 * q + 8 * k
                # boundary trim: k=0 keeps h_off 8..15, k=6 keeps 0..7
                if k == 0:
                    rlo, rhi, cls0 = r0 + 8, r0 + 16, 64
                elif k == NRB - 1:
                    rlo, rhi, cls0 = r0, r0 + 8, 0
                else:
                    rlo, rhi, cls0 = r0, r0 + 16, None
                mw = rhi - rlo  # moving rows (8 or 16)
                nmv = mw * 8
                for g in range(4):  # groups of 8 w-tiles per 2-bank PSUM tile
                    pt = psp.tile([128, nmv * 8], dt.float32, tag="ps1")
                    for u in range(8):
                        w0 = g * 8 + u
                        lhsT = f2s[:, 64 * w0 : 64 * w0 + 128]     # [128,128]
                        rhs = f1v[:, rlo:rhi, 8 * w0 : 8 * w0 + T]
                        nc.tensor.matmul(
                            pt[:, u * nmv : (u + 1) * nmv],
                            lhsT,
                            rhs,
                            start=True,
                            stop=True,
                        )
                    t0 = k * 32 + g * 8
                    src = pt[:, :].rearrange("p (u c) -> p c u", u=8)
                    if cls0 is None:
                        dst = stv2[:, :, t0 : t0 + 8]
                    else:
                        dst = stv2[:, cls0 : cls0 + 64, t0 : t0 + 8]
                    copy(dst, src)

            # ---- pass 2: selection matmuls + per-phase out stores ----
            for t in range(8, 16):
                ot = outp.tile([81, 6 * W], dt.float32, tag="outt")
                otv = ot[:, :].rearrange("p (a b c) -> p a b c", a=6, b=32)
                for gr in range(2):  # 4 wl per PSUM tile -> 1 grouped copy
                    p2 = psp.tile([128, 1024], dt.float32, tag="ps1")
                    for s in range(4):
                        wl = 4 * gr + s
                        clsA = t * 8 + wl
                        clsB = (t - 8) * 8 + wl
                        dst2 = p2[0:81, 256 * s : 256 * s + 192]
                        cA = _sel_col(t, wl)
                        cB = _sel_col(t - 8, wl)
                        nc.tensor.matmul(
                            dst2,
                            selb[:, cA : cA + 81],
                            stv2[:, clsA, 0:192],   # tiles k=0..5
                            start=True,
                            stop=False,
                        )
                        nc.tensor.matmul(
                            dst2,
                            selb[:, cB : cB + 81],
                            stv2[:, clsB, 32:224],  # tiles k=1..6
                            start=False,
                            stop=True,
                        )
                    dst = otv[:, :, :, 4 * gr : 4 * gr + 4]  # [81, 6, 32, 4]
                    src = (
                        p2[0:81, :]
                        .rearrange("p (s x) -> p s x", s=4)[:, :, 0:192]
                        .rearrange("p s (a b) -> p a b s", a=6)
                    )
                    copy(dst, src)
                # contiguous permuted store: out col = ((q*8+tt)*6 + a)*256 + w
                nc.sync.dma_start(
                    out[:, (q * 8 + t - 8) * 6 * W : (q * 8 + t - 7) * 6 * W],
                    ot[:, :],
                )


def _build_nc():
    import concourse.mybir as mybir
    import concourse.tile as tile
    from concourse import bacc

    dt = mybir.dt
    nc = bacc.Bacc("TRN2", target_bir_lowering=False, debug=False)
    f1 = nc.dram_tensor("f1", [C, OH * W], dt.float16, kind="ExternalInput")
    f2 = nc.dram_tensor(
        "f2", [C, NSLAB * F2W * 8], dt.float16, kind="ExternalInput"
    )
    selt = nc.dram_tensor("sel", [C, 128 * 81], dt.float16, kind="ExternalInput")
    out = nc.dram_tensor("out", [D, OH * W], dt.float32, kind="ExternalOutput")
    with tile.TileContext(nc) as tc:
        _emit(tc, f1[:, :], f2[:, :], selt[:, :], out[:, :])
    nc.finalize()
    return nc


def _shard_inputs(feat1, feat2):
    sel = _build_sel()
    in_maps = []
    for core in range(NCORES):
        b, half = core // 2, core % 2
        h0 = half * OH
        f1s = (feat1[b, :, h0 : h0 + OH, :] * SCALE).astype(np.float16)
        f2pad = np.zeros((C, OH + 8, F2W), np.float16)
        lo, hi = h0 - 4, h0 + OH + 4
        slo, shi = max(lo, 0), min(hi, H)
        f2pad[:, slo - lo : shi - lo, 4 : 4 + W] = feat2[b, :, slo:shi, :].astype(
            np.float16
        )
        # slab g -> f2pad rows [8g, 8g+8) (= global rows 8g-4..8g+4),
        # transposed to [C, w, r] so each (16 w x 8 r) weight window is
        # contiguous.
        slabs = np.zeros((C, NSLAB, F2W, 8), np.float16)
        for g in range(NSLAB):
            slabs[:, g] = f2pad[:, 8 * g : 8 * g + 8, :].transpose(0, 2, 1)
        in_maps.append(
            {
                "f1": np.ascontiguousarray(f1s.reshape(C, OH * W)),
                "f2": np.ascontiguousarray(slabs.reshape(C, -1)),
                "sel": sel,
            }
        )
    return in_maps


def _unshard_out(arr):
    """[D, OH*W] permuted (q, tt, a, w) device layout -> [D, OH, W]."""
    return (
        arr.reshape(D, 2, 8, 6, W)
        .transpose(0, 1, 3, 2, 4)
        .reshape(D, OH, W)
    )


def kernel(feat1, feat2):
    feat1 = np.asarray(feat1, dtype=np.float32)
    feat2 = np.asarray(feat2, dtype=np.float32)
    from concourse.bass_utils import run_bass_kernel_spmd

    nc = _build_nc()
    in_maps = _shard_inputs(feat1, feat2)
    res = run_bass_kernel_spmd(nc, in_maps, list(range(NCORES)))
    full = np.zeros((B, D, H, W), np.float32)
    for core in range(NCORES):
        b, half = core // 2, core % 2
        full[b, :, half * OH : (half + 1) * OH, :] = _unshard_out(
            res.results[core]["out"].astype(np.float32)
        )
    return full
